# revision 1
# baseline (speedup 1.0000x reference)
"""Multi-head attention (B=4, S=2048, E=1024, H=16) on 8 TRN2 NeuronCores.

Sharding: core c handles (batch b = c//2, query S-half = c%2) -> 1024 query
rows per core; all 16 heads stay on-core. K/V projections for the full
sequence of a batch are computed (duplicated) on both cores of that batch,
which avoids any cross-core collective. The attention-score layout keeps
keys on partitions and queries on the free dim ([k, q]) so no transposes of
the probability matrix are ever needed; the softmax denominator is obtained
for free by appending a ones-column to V in the PV matmul, and
normalization happens after PV (64x cheaper than normalizing probs).

All matmuls run in float32r (fp32 bits, ~13-bit effective mantissa on the
PE, full bf16-rate at moving-dim >= 256). Activations and weights are
shipped host-transposed ([e, m] / [e, o]) and declared float32r in DRAM so
they DMA straight into matmul operands with no on-chip transposes or
rounding passes. The mask input is all-ones per the problem spec, so
`where(mask==0, -1e9)` is a no-op and the mask is not shipped to the device.
"""

import sys

sys.path.insert(0, "/opt/trn_rl_repo")

from contextlib import ExitStack

import numpy as np

import concourse.bass as bass
import concourse.bacc as bacc
import concourse.tile as tile
from concourse import mybir
from concourse.bass_utils import run_bass_kernel_spmd

P = 128
DH = 64

F32 = mybir.dt.float32
F32R = mybir.dt.float32r


def build_core_program(M=1024, S=2048, E=1024, H=16, QT=512, repeat=1):
    """One core's program: full MHA for M query rows against S keys.

    Inputs (per core): transposed activations xqt [E,M], xkt [E,S],
    xvt [E,S]; pre-transposed weights wqt/wkt/wvt/wot [E,E] (= W.T, i.e.
    [e_in, e_out]); biases [E]. Output: o [M,E].
    """
    assert E % P == 0 and S % (2 * P) == 0 and M % P == 0 and H % 2 == 0
    assert H * DH == E
    EC = E // P  # e chunks (contraction chunks for projections)
    MC = M // P
    SC = S // P
    HP = H // 2  # head pairs; also number of 128-wide o-chunks
    assert HP == EC
    QT = min(QT, M)
    NQT = M // QT

    nc = bacc.Bacc("TRN2", target_bir_lowering=False, debug=False)

    xqt = nc.dram_tensor("xqt", [E, M], F32R, kind="ExternalInput")
    xkt = nc.dram_tensor("xkt", [E, S], F32R, kind="ExternalInput")
    xvt = nc.dram_tensor("xvt", [E, S], F32R, kind="ExternalInput")
    wqt = nc.dram_tensor("wqt", [E, E], F32R, kind="ExternalInput")
    wkt = nc.dram_tensor("wkt", [E, E], F32R, kind="ExternalInput")
    wvt = nc.dram_tensor("wvt", [E, E], F32R, kind="ExternalInput")
    wot = nc.dram_tensor("wot", [E, E], F32R, kind="ExternalInput")
    bq = nc.dram_tensor("bq", [E], F32, kind="ExternalInput")
    bk = nc.dram_tensor("bk", [E], F32, kind="ExternalInput")
    bv = nc.dram_tensor("bv", [E], F32, kind="ExternalInput")
    bo = nc.dram_tensor("bo", [E], F32, kind="ExternalInput")
    out = nc.dram_tensor("o", [M, E], F32, kind="ExternalOutput")

    # DRAM scratch for staged K^T and V (SBUF cannot hold them all).
    khT_d = nc.dram_tensor("khT_d", [E, S], F32R, kind="Internal")
    # V staged with a ones-column per head: [S, H*65]
    vh_d = nc.dram_tensor("vh_d", [S, H * 65], F32R, kind="Internal")

    scale = 1.0 / np.sqrt(DH)

    def load_weights(pool, w_dram, prefix):
        # One [128, EC*E] tile per weight from a double-buffered tag: the
        # 3rd/4th weights reuse the 1st/2nd slots, so their DMAs naturally
        # wait for the previous phase's last matmul and prefetch during the
        # intervening phase.
        w = pool.tile([P, EC * E], F32R, tag="w", name=f"{prefix}")
        for e in range(EC):
            nc.sync.dma_start(
                out=w[:, e * E : (e + 1) * E], in_=w_dram[e * P : (e + 1) * P, :]
            )
        return [w[:, e * E : (e + 1) * E] for e in range(EC)]

    with tile.TileContext(nc) as tc, ExitStack() as ctx:
        consts = ctx.enter_context(tc.tile_pool(name="consts", bufs=1))
        persist = ctx.enter_context(tc.tile_pool(name="persist", bufs=1))

        # --- constants ---
        # per-partition bias layouts for transposed-output projections:
        # bX_sb[p, c] = bX[c*128 + p]
        bq_sb = consts.tile([P, EC], F32)
        bk_sb = consts.tile([P, EC], F32)
        nc.sync.dma_start(out=bq_sb, in_=bq.ap().rearrange("(c p) -> p c", p=P))
        nc.sync.dma_start(out=bk_sb, in_=bk.ap().rearrange("(c p) -> p c", p=P))
        # free-dim broadcast tiles for bv / bo
        bv_bc = consts.tile([P, E], F32)
        bo_bc = consts.tile([P, E], F32)
        nc.gpsimd.dma_start(
            out=bv_bc, in_=bass.AP(tensor=bv, offset=0, ap=[[0, P], [1, E]])
        )
        nc.gpsimd.dma_start(
            out=bo_bc, in_=bass.AP(tensor=bo, offset=0, ap=[[0, P], [1, E]])
        )
        # ones row (at partition 64) for the K=1 denominator-broadcast matmul
        ones_f = consts.tile([P, DH], F32)
        nc.vector.memset(ones_f, 1.0)
        ones_r = consts.tile([P, DH], F32R)
        nc.vector.tensor_copy(ones_r, ones_f)

        for rep in range(repeat):
            pw_stk = ExitStack()
            pw = pw_stk.enter_context(tc.tile_pool(name=f"pW{rep}", bufs=2))
            # persistent across phases
            qhT = [persist.tile([P, M], F32R, tag=f"qhT{c}", name=f"qhT{c}") for c in range(EC)]
            concatT = [persist.tile([P, M], F32R, tag=f"ccT{c}", name=f"ccT{c}") for c in range(EC)]

            # ------------- Phase 1: Q projection (output transposed) -------
            # qhT[o, m] = (xq @ Wq.T).T + bq ; qhT chunk c lives on
            # partitions o = c*128..c*128+127 (heads 2c, 2c+1 stacked).
            with tc.tile_pool(name=f"pQ{rep}", bufs=1) as pq, \
                 tc.tile_pool(name=f"pQps2{rep}", bufs=6, space="PSUM") as pqps2:
                xTq = pq.tile([P, EC * M], F32R, tag="xTq", name="xTq")
                wq = pw.tile([P, EC * E], F32R, tag="w", name="wq")
                for e in range(EC):
                    nc.sync.dma_start(
                        out=wq[:, e * E : (e + 1) * E],
                        in_=wqt[e * P : (e + 1) * P, :],
                    )
                    nc.sync.dma_start(
                        out=xTq[:, e * M : (e + 1) * M],
                        in_=xqt[e * P : (e + 1) * P, :],
                    )
                wq_t = [wq[:, e * E : (e + 1) * E] for e in range(EC)]
                # prefetch K weights during the Q matmuls
                wk_t = load_weights(pw, wkt, "wk")
                for c in range(EC):
                    for t in range(M // QT):
                        ps = pqps2.tile([P, QT], F32, tag="qps", name="qps")
                        for e in range(EC):
                            nc.tensor.matmul(
                                ps,
                                wq_t[e][:, c * P : (c + 1) * P],
                                xTq[:, e * M + t * QT : e * M + (t + 1) * QT],
                                start=(e == 0),
                                stop=(e == EC - 1),
                            )
                        nc.vector.tensor_scalar_add(
                            qhT[c][:, t * QT : (t + 1) * QT], ps, bq_sb[:, c : c + 1]
                        )

            # ------------- Phase 2: K projection -> khT_d scratch ----------
            ST = 512  # s-tile width
            with tc.tile_pool(name=f"pKx{rep}", bufs=1) as pkx, \
                 tc.tile_pool(name=f"pKs{rep}", bufs=4) as pks, \
                 tc.tile_pool(name=f"pVx{rep}", bufs=1) as pvx, \
                 tc.tile_pool(name=f"pVs{rep}", bufs=2) as pvs, \
                 tc.tile_pool(name=f"pKps2{rep}", bufs=4, space="PSUM") as pkps2, \
                 tc.tile_pool(name=f"pVps2{rep}", bufs=4, space="PSUM") as pvps2:
                for st in range(S // ST):
                    xTk = pkx.tile([P, EC * ST], F32R, tag="xTk", name="xTk")
                    for e in range(EC):
                        nc.sync.dma_start(
                            out=xTk[:, e * ST : (e + 1) * ST],
                            in_=xkt[e * P : (e + 1) * P, st * ST : (st + 1) * ST],
                        )
                    if st == 0:
                        # prefetch V weights during the K matmuls
                        wv_t = load_weights(pw, wvt, "wv")
                    for c in range(EC):
                        ps = pkps2.tile([P, ST], F32, tag="kps", name="kps")
                        for e in range(EC):
                            nc.tensor.matmul(
                                ps,
                                wk_t[e][:, c * P : (c + 1) * P],
                                xTk[:, e * ST : (e + 1) * ST],
                                start=(e == 0),
                                stop=(e == EC - 1),
                            )
                        stgo = pks.tile([P, ST], F32R, tag="kout", name="kout")
                        nc.vector.tensor_scalar_add(stgo, ps, bk_sb[:, c : c + 1])
                        nc.sync.dma_start(
                            out=khT_d[c * P : (c + 1) * P, st * ST : (st + 1) * ST],
                            in_=stgo,
                        )
                    xTv = pvx.tile([P, EC * ST], F32R, tag="xTv", name="xTv")
                    for e in range(EC):
                        nc.sync.dma_start(
                            out=xTv[:, e * ST : (e + 1) * ST],
                            in_=xvt[e * P : (e + 1) * P, st * ST : (st + 1) * ST],
                        )
                    if st == min(1, S // ST - 1):
                        # prefetch O weights during the V matmuls
                        wo_t = load_weights(pw, wot, "wo")
                    for ss in range(ST // P):
                        sc = st * (ST // P) + ss
                        vst = pvs.tile([P, H * 65], F32R, tag="vst", name="vst")
                        vst3 = vst.rearrange("p (h d) -> p h d", d=65)
                        # ones column per head
                        nc.vector.tensor_copy(vst3[:, :, 64], ones_f[:, 0:H])
                        for oh in range(E // 512):
                            ps = pvps2.tile([P, 512], F32, tag="vps", name="vps")
                            for e in range(EC):
                                nc.tensor.matmul(
                                    ps,
                                    xTv[:, e * ST + ss * P : e * ST + (ss + 1) * P],
                                    wv_t[e][:, oh * 512 : (oh + 1) * 512],
                                    start=(e == 0),
                                    stop=(e == EC - 1),
                                )
                            # evac + bias into strided head slots
                            nc.vector.tensor_add(
                                vst3[:, oh * 8 : (oh + 1) * 8, 0:64],
                                ps.rearrange("p (h d) -> p h d", d=DH),
                                bv_bc[:, oh * 512 : (oh + 1) * 512].rearrange(
                                    "p (h d) -> p h d", d=DH
                                ),
                            )
                        nc.sync.dma_start(
                            out=vh_d[sc * P : (sc + 1) * P, :], in_=vst
                        )

            # ------------- Phase 4: attention per head pair ----------------
            if True:
                with tc.tile_pool(name=f"pA{rep}", bufs=2) as pa, \
                     tc.tile_pool(name=f"pAp{rep}", bufs=3) as pap, \
                     tc.tile_pool(name=f"pAd{rep}", bufs=2) as pad, \
                     tc.tile_pool(name=f"pAsc{rep}", bufs=3, space="PSUM") as pasc, \
                     tc.tile_pool(name=f"pAat{rep}", bufs=1, space="PSUM") as paat:
                    for p in range(HP):
                        khT = pa.tile([P, S], F32R, tag="khT", name="khT")
                        nc.sync.dma_start(out=khT, in_=khT_d[p * P : (p + 1) * P, :])
                        vh = pa.tile([P, SC * 130], F32R, tag="vh", name="vh")
                        nc.sync.dma_start(
                            out=vh.rearrange("p (s c) -> p s c", c=130),
                            in_=vh_d.ap().rearrange("(s p) o -> p s o", p=P)[
                                :, :, p * 130 : (p + 1) * 130
                            ],
                        )
                        for t in range(NQT):
                            att = [
                                paat.tile([P, QT], F32, tag=f"att{j}", name=f"att{j}")
                                for j in range(2)
                            ]
                            for g in range(SC // 2):
                                for j in range(2):  # head within pair
                                    sc_ps = pasc.tile(
                                        [P, 2 * QT], F32, tag="scps", name="scps"
                                    )
                                    for u in range(2):  # k-chunk within group
                                        kc = 2 * g + u
                                        nc.tensor.matmul(
                                            sc_ps[:, u * QT : (u + 1) * QT],
                                            khT[j * DH : (j + 1) * DH, kc * P : (kc + 1) * P],
                                            qhT[p][j * DH : (j + 1) * DH, t * QT : (t + 1) * QT],
                                            start=True,
                                            stop=True,
                                        )
                                    pr = pap.tile(
                                        [P, 2 * QT], F32R, tag="probs", name="probs"
                                    )
                                    nc.scalar.activation(
                                        pr, sc_ps, mybir.ActivationFunctionType.Exp,
                                        scale=float(scale),
                                    )
                                    for u in range(2):
                                        kc = 2 * g + u
                                        nc.tensor.matmul(
                                            att[j][0:65, :],
                                            vh[:, kc * 130 + j * 65 : kc * 130 + (j + 1) * 65],
                                            pr[:, u * QT : (u + 1) * QT],
                                            start=(kc == 0),
                                            stop=(kc == SC - 1),
                                        )
                            # denominators -> broadcast -> reciprocal ->
                            # normalize. den lives at PSUM partition 64 (the
                            # ones-column row of the PV output); DVE is
                            # lane-aligned so it is copied out at partition 64
                            # and broadcast to partitions 0..63 via a K=1
                            # matmul whose operands sit at partition 64.
                            for j in range(2):
                                den = pad.tile(
                                    [65, QT], F32R, tag=f"den{j}", name=f"den{j}"
                                )
                                nc.vector.tensor_copy(den[64:65, :], att[j][64:65, :])
                                dbc = pasc.tile([DH, QT], F32, tag="scps", name="dbc")
                                nc.tensor.matmul(
                                    dbc, ones_r[64:65, :], den[64:65, :],
                                    start=True, stop=True,
                                )
                                rec = pad.tile(
                                    [DH, QT], F32, tag=f"rec{j}", name=f"rec{j}"
                                )
                                nc.vector.reciprocal_approx_fast(rec, dbc)
                                if j == 0:
                                    nc.vector.tensor_mul(
                                        concatT[p][0:DH, t * QT : (t + 1) * QT],
                                        att[j][0:DH, :],
                                        rec,
                                    )
                                else:
                                    # head 1 computes at partitions 0..63;
                                    # shift to concatT partitions 64..127 via
                                    # SBUF->SBUF DMA
                                    tmp1 = pad.tile(
                                        [DH, QT], F32R, tag="tmp1", name="tmp1"
                                    )
                                    nc.vector.tensor_mul(tmp1, att[j][0:DH, :], rec)
                                    nc.sync.dma_start(
                                        out=concatT[p][DH:P, t * QT : (t + 1) * QT],
                                        in_=tmp1,
                                    )

                # ------------- Phase 5: O projection -----------------------
                with tc.tile_pool(name=f"pOn{rep}", bufs=3) as pon, \
                     tc.tile_pool(name=f"pOps{rep}", bufs=3, space="PSUM") as pops:
                    for mc in range(MC):
                        for nh in range(E // 512):
                            ps = pops.tile([P, 512], F32, tag="ops", name="ops")
                            for c in range(EC):
                                nc.tensor.matmul(
                                    ps,
                                    concatT[c][:, mc * P : (mc + 1) * P],
                                    wo_t[c][:, nh * 512 : (nh + 1) * 512],
                                    start=(c == 0),
                                    stop=(c == EC - 1),
                                )
                            ob = pon.tile([P, 512], F32, tag="ob", name="ob")
                            nc.vector.tensor_add(
                                ob, ps, bo_bc[:, nh * 512 : (nh + 1) * 512]
                            )
                            nc.sync.dma_start(
                                out=out[mc * P : (mc + 1) * P, nh * 512 : (nh + 1) * 512],
                                in_=ob,
                            )
            pw_stk.close()

    nc.compile()
    return nc


_PROGRAM_CACHE = {}


def _get_program(key=(1024, 2048, 1024, 16)):
    if key not in _PROGRAM_CACHE:
        _PROGRAM_CACHE[key] = build_core_program(*key)
    return _PROGRAM_CACHE[key]


_LAST_RESULTS = None


def make_in_maps(q, k, v, Wq, bq, Wk, bk, Wv, bv, Wo, bo, n_cores=8):
    B, S, E = q.shape
    halves = n_cores // B
    MS = S // halves  # query rows per core
    shared = {
        "wqt": np.ascontiguousarray(np.asarray(Wq).T),
        "wkt": np.ascontiguousarray(np.asarray(Wk).T),
        "wvt": np.ascontiguousarray(np.asarray(Wv).T),
        "wot": np.ascontiguousarray(np.asarray(Wo).T),
        "bq": np.asarray(bq), "bk": np.asarray(bk),
        "bv": np.asarray(bv), "bo": np.asarray(bo),
    }
    kT = [np.ascontiguousarray(np.asarray(k[b]).T) for b in range(B)]
    vT = [np.ascontiguousarray(np.asarray(v[b]).T) for b in range(B)]
    in_maps = []
    for c in range(n_cores):
        b, h = divmod(c, halves)
        in_maps.append({
            "xqt": np.ascontiguousarray(np.asarray(q[b, h * MS : (h + 1) * MS, :]).T),
            "xkt": kT[b],
            "xvt": vT[b],
            **shared,
        })
    return in_maps


def kernel(q, k, v, mask, Wq, bq, Wk, bk, Wv, bv, Wo, bo, **run_kwargs):
    q = np.asarray(q, dtype=np.float32)
    k = np.asarray(k, dtype=np.float32)
    v = np.asarray(v, dtype=np.float32)
    B, S, E = q.shape
    n_cores = 8
    halves = n_cores // B
    MS = S // halves
    nc = _get_program((MS, S, E, 16))
    in_maps = make_in_maps(q, k, v, Wq, bq, Wk, bk, Wv, bv, Wo, bo, n_cores)
    res = run_bass_kernel_spmd(nc, in_maps, core_ids=list(range(n_cores)), **run_kwargs)
    global _LAST_RESULTS
    _LAST_RESULTS = res
    out = np.empty((B, S, E), dtype=np.float32)
    for c in range(n_cores):
        b, h = divmod(c, halves)
        out[b, h * MS : (h + 1) * MS, :] = res.results[c]["o"]
    return out

